# Initial kernel scaffold
#
"""Trainium2 Bass kernel for CodebookMapper (vq_codebook).

Full-input contract: kernel(x[32768,512] f32, codebook[8192,512] f32) ->
quantized[32768,512] f32, computing
    xn   = l2norm(x, axis=1)
    sims = xn @ codebook.T / 0.07
    soft = softmax(sims, axis=1)
    out  = soft @ codebook

Sharding: data-parallel over rows of x across 8 NeuronCores; codebook
replicated. Each core runs an identical NEFF on its 4096-row shard.

Per-core dataflow (bf16 matmuls, fp32 accumulation):
  setup: load codebook, cast to bf16 (cb_n, [k,d] layout), and build the
         transposed copy cb_t ([d,k] layout) with PE-mode transposes.
  per 128-row tile of x:
    1. normalize rows in fp32 (Square+accum on ACT, rsqrt via Sqrt+recip),
       cast to bf16
    2. PE-transpose xn -> xnT (lhsT for GEMM1)
    3. GEMM1: sims chunk [128,512] = xnT.T @ cb_t chunk, accumulated over
       d in PSUM; ACT applies exp(sims/tau) PSUM->SBUF bf16, with the
       per-row sum of each chunk accumulated for free via accum_out
    4. GEMM2: PE-transpose each exp tile [128,128] -> lhsT, accumulate
       q_unnorm [128,512] = sum_k expT.T @ cb_n[k] in a single PSUM bank
    5. softmax normalization folded into the epilogue:
       out = q_unnorm * (1/rowsum), DMA to DRAM

exp needs no max-subtraction: |logits| <= 1/tau = 14.3 so exp is in
[6e-7, 1.6e6], comfortably inside fp32/bf16 range.
"""

import numpy as np

import concourse.bass as bass
import concourse.tile as tile
from concourse import bacc, mybir
from concourse.bass_utils import run_bass_kernel_spmd
from concourse.masks import make_identity

N_CORES = 8
K_FULL = 8192
D_FULL = 512
TAU = 0.07

F32 = mybir.dt.float32
BF16 = mybir.dt.bfloat16
AF = mybir.ActivationFunctionType
ALU = mybir.AluOpType


def _build_kernel(tc: tile.TileContext, out_ap, x_ap, cb_ap, n_local, k, d):
    nc = tc.nc
    P = 128
    KT = k // P          # 64  k-tiles (codebook rows per partition-chunk)
    DT = d // P          # 4   d-tiles
    NCH = k // 512       # 16  512-wide chunks of the sims row
    MT = n_local // P    # 32  row tiles per core

    persist = tc.alloc_tile_pool(name="persist", bufs=1)
    stage = tc.alloc_tile_pool(name="stage", bufs=3)
    io_pool = tc.alloc_tile_pool(name="io", bufs=2)
    exp_pool = tc.alloc_tile_pool(name="exp", bufs=2)
    st_pool = tc.alloc_tile_pool(name="st", bufs=4)
    small = tc.alloc_tile_pool(name="small", bufs=4)
    psum_t = tc.alloc_tile_pool(name="psum_t", bufs=3, space="PSUM")
    psum_g1 = tc.alloc_tile_pool(name="psum_g1", bufs=2, space="PSUM")
    psum_q = tc.alloc_tile_pool(name="psum_q", bufs=2, space="PSUM")

    ident = persist.tile([P, P], BF16)
    make_identity(nc, ident)

    # codebook, natural [k, d] layout, partition-chunked over k, bf16
    cb_n = persist.tile([P, KT, d], BF16)
    # codebook transposed to [d, k], partition-chunked over d, bf16
    cb_t = persist.tile([P, DT, k], BF16)

    for ko in range(KT):
        cst = stage.tile([P, d], F32)
        nc.sync.dma_start(cst, cb_ap[ko * P:(ko + 1) * P, :])
        nc.gpsimd.tensor_copy(cb_n[:, ko, :], cst)
        for dd in range(DT):
            tps = psum_t.tile([P, P], BF16, tag="pst")
            nc.tensor.transpose(tps, cb_n[:, ko, dd * P:(dd + 1) * P], ident)
            nc.vector.tensor_copy(cb_t[:, dd, ko * P:(ko + 1) * P], tps)

    inv_tau = float(1.0 / TAU)

    for m in range(MT):
        row0 = m * P
        # ---- load + normalize ----
        x_t = io_pool.tile([P, d], F32)
        nc.sync.dma_start(x_t, x_ap[row0:row0 + P, :])
        sq = io_pool.tile([P, d], F32)
        ss = small.tile([P, 1], F32)
        nc.scalar.activation(out=sq, in_=x_t, func=AF.Square, accum_out=ss)
        nrm = small.tile([P, 1], F32)
        nc.scalar.sqrt(nrm, ss)
        rstd = small.tile([P, 1], F32)
        nc.vector.reciprocal(rstd, nrm)
        xn_b = io_pool.tile([P, d], BF16)
        nc.vector.tensor_scalar_mul(xn_b, x_t, rstd)

        # ---- transpose xn -> lhsT for GEMM1 ----
        xnT = io_pool.tile([P, DT, P], BF16)
        for dd in range(DT):
            xps = psum_t.tile([P, P], BF16, tag="pst")
            nc.tensor.transpose(xps, xn_b[:, dd * P:(dd + 1) * P], ident)
            nc.scalar.copy(xnT[:, dd, :], xps)

        # ---- GEMM1 + exp ----
        exp_b = exp_pool.tile([P, k], BF16)
        parts = small.tile([P, NCH], F32)
        for n in range(NCH):
            ps = psum_g1.tile([P, 512], F32)
            for dd in range(DT):
                nc.tensor.matmul(
                    ps,
                    xnT[:, dd, :],
                    cb_t[:, dd, n * 512:(n + 1) * 512],
                    start=(dd == 0),
                    stop=(dd == DT - 1),
                )
            nc.scalar.activation(
                out=exp_b[:, n * 512:(n + 1) * 512],
                in_=ps,
                func=AF.Exp,
                scale=inv_tau,
                accum_out=parts[:, n:n + 1],
            )

        rs = small.tile([P, 1], F32)
        nc.vector.tensor_reduce(rs, parts, axis=mybir.AxisListType.X, op=ALU.add)
        rr = small.tile([P, 1], F32)
        nc.vector.reciprocal(rr, rs)

        # ---- GEMM2: q_unnorm = exp @ cb ----
        qacc = psum_q.tile([P, d], F32)
        for kk in range(KT):
            pst = psum_t.tile([P, P], BF16, tag="pst")
            nc.tensor.transpose(pst, exp_b[:, kk * P:(kk + 1) * P], ident)
            st = st_pool.tile([P, P], BF16)
            nc.vector.tensor_copy(st, pst)
            nc.tensor.matmul(
                qacc, st, cb_n[:, kk, :], start=(kk == 0), stop=(kk == KT - 1)
            )

        # ---- epilogue: fold softmax denominator into output scale ----
        o_sb = io_pool.tile([P, d], F32)
        nc.vector.tensor_scalar_mul(o_sb, qacc, rr)
        nc.sync.dma_start(out_ap[row0:row0 + P, :], o_sb)


def build_bass(n_local, k=K_FULL, d=D_FULL, n_cores=N_CORES):
    nc = bacc.Bacc(
        "TRN2",
        target_bir_lowering=False,
        debug=False,
        num_devices=n_cores,
    )
    x_ap = nc.dram_tensor("x", [n_local, d], F32, kind="ExternalInput").ap()
    cb_ap = nc.dram_tensor("codebook", [k, d], F32, kind="ExternalInput").ap()
    out_ap = nc.dram_tensor("out", [n_local, d], F32, kind="ExternalOutput").ap()
    with tile.TileContext(nc) as tc:
        _build_kernel(tc, out_ap, x_ap, cb_ap, n_local, k, d)
    nc.compile()
    return nc


_NC_CACHE = {}


def _get_nc(n_local, k, d, n_cores):
    key = (n_local, k, d, n_cores)
    if key not in _NC_CACHE:
        _NC_CACHE[key] = build_bass(n_local, k, d, n_cores)
    return _NC_CACHE[key]


def run_sharded(x, codebook, trace=False):
    n, d = x.shape
    k = codebook.shape[0]
    assert n % N_CORES == 0
    n_local = n // N_CORES
    nc = _get_nc(n_local, k, d, N_CORES)
    cb = np.ascontiguousarray(codebook, dtype=np.float32)
    in_maps = [
        {
            "x": np.ascontiguousarray(x[i * n_local:(i + 1) * n_local],
                                      dtype=np.float32),
            "codebook": cb,
        }
        for i in range(N_CORES)
    ]
    res = run_bass_kernel_spmd(
        nc, in_maps, core_ids=list(range(N_CORES)), trace=trace
    )
    out = np.concatenate([r["out"] for r in res.results], axis=0)
    return out, res


def kernel(x, codebook):
    out, _ = run_sharded(x, codebook, trace=False)
    return out


# revision 11
# speedup vs baseline: 1.1798x; 1.1798x over previous
"""Trainium2 Bass kernel for CodebookMapper (vq_codebook).

Full-input contract: kernel(x[32768,512] f32, codebook[8192,512] f32) ->
quantized[32768,512] f32, computing
    xn   = l2norm(x, axis=1)
    sims = xn @ codebook.T / 0.07
    soft = softmax(sims, axis=1)
    out  = soft @ codebook

Sharding: data-parallel over rows of x across 8 NeuronCores; codebook
replicated. Each core runs an identical NEFF on its 4096-row shard.

Per-core dataflow (bf16 matmuls, fp32 accumulation):
  setup: load codebook, cast to bf16 (cb_n, [k,d] layout), and build the
         transposed copy cb_t ([d,k] layout) with PE-mode transposes.
  per 128-row tile of x:
    1. normalize rows in fp32 (Square+accum on ACT, rsqrt via Sqrt+recip),
       cast to bf16
    2. PE-transpose xn -> xnT (lhsT for GEMM1)
    3. GEMM1: sims chunk [128,512] = xnT.T @ cb_t chunk, accumulated over
       d in PSUM; ACT applies exp(sims/tau) PSUM->SBUF bf16, with the
       per-row sum of each chunk accumulated for free via accum_out
    4. GEMM2: PE-transpose each exp tile [128,128] -> lhsT, accumulate
       q_unnorm [128,512] = sum_k expT.T @ cb_n[k] in a single PSUM bank
    5. softmax normalization folded into the epilogue:
       out = q_unnorm * (1/rowsum), DMA to DRAM

exp needs no max-subtraction: |logits| <= 1/tau = 14.3 so exp is in
[6e-7, 1.6e6], comfortably inside fp32/bf16 range.
"""

import numpy as np

import concourse.bass as bass
import concourse.tile as tile
from concourse import bacc, mybir
from concourse.bass_utils import run_bass_kernel_spmd
from concourse.masks import make_identity

N_CORES = 8
K_FULL = 8192
D_FULL = 512
TAU = 0.07

F32 = mybir.dt.float32
BF16 = mybir.dt.bfloat16
AF = mybir.ActivationFunctionType
ALU = mybir.AluOpType


USE_DMA_TRANSPOSE = True


def _build_kernel(tc: tile.TileContext, out_ap, x_ap, cb_ap, n_local, k, d,
                  reps=1):
    from contextlib import ExitStack

    with ExitStack() as ctx:
        if reps > 1:
            # Timing harness: loop the whole kernel on-device so host /
            # axon dispatch overhead can be differenced away.
            with tc.For_i(0, reps, 1):
                _build_kernel_inner(ctx, tc, out_ap, x_ap, cb_ap, n_local, k, d)
        else:
            _build_kernel_inner(ctx, tc, out_ap, x_ap, cb_ap, n_local, k, d)


def _build_kernel_inner(ctx, tc, out_ap, x_ap, cb_ap, n_local, k, d):
    nc = tc.nc
    P = 128
    KT = k // P          # 64  k-tiles (codebook rows per partition-chunk)
    DT = d // P          # 4   d-tiles
    NCH = k // 512       # 16  512-wide chunks of the sims row
    MT = n_local // P    # 32  row tiles per core

    persist = ctx.enter_context(tc.tile_pool(name="persist", bufs=1))
    stage = ctx.enter_context(tc.tile_pool(name="stage", bufs=3))
    io_pool = ctx.enter_context(tc.tile_pool(name="io", bufs=2))
    exp_pool = ctx.enter_context(tc.tile_pool(name="exp", bufs=2))
    st_pool = ctx.enter_context(tc.tile_pool(name="st", bufs=8))
    small = ctx.enter_context(tc.tile_pool(name="small", bufs=4))
    psum_t = ctx.enter_context(tc.tile_pool(name="psum_t", bufs=3, space="PSUM"))
    psum_g1 = ctx.enter_context(tc.tile_pool(name="psum_g1", bufs=2, space="PSUM"))
    psum_q = ctx.enter_context(tc.tile_pool(name="psum_q", bufs=2, space="PSUM"))

    ident = persist.tile([P, P], BF16)
    make_identity(nc, ident)

    # codebook, natural [k, d] layout, partition-chunked over k, bf16
    cb_n = persist.tile([P, KT, d], BF16)
    # codebook transposed to [d, k], partition-chunked over d, bf16
    cb_t = persist.tile([P, DT, k], BF16)

    for ko in range(KT):
        cst = stage.tile([P, d], F32)
        nc.sync.dma_start(cst, cb_ap[ko * P:(ko + 1) * P, :])
        nc.gpsimd.tensor_copy(cb_n[:, ko, :], cst)
        for dd in range(DT):
            if USE_DMA_TRANSPOSE:
                nc.sync.dma_start(
                    cb_t[:, dd, ko * P:(ko + 1) * P],
                    cb_n[:, ko, dd * P:(dd + 1) * P],
                    transpose=True,
                )
            else:
                tps = psum_t.tile([P, P], BF16, tag="pst")
                nc.tensor.transpose(tps, cb_n[:, ko, dd * P:(dd + 1) * P], ident)
                nc.vector.tensor_copy(cb_t[:, dd, ko * P:(ko + 1) * P], tps)

    inv_tau = float(1.0 / TAU)

    for m in range(MT):
        row0 = m * P
        # ---- load + normalize ----
        x_t = io_pool.tile([P, d], F32)
        nc.sync.dma_start(x_t, x_ap[row0:row0 + P, :])
        sq = io_pool.tile([P, d], F32)
        ss = small.tile([P, 1], F32)
        nc.scalar.activation(out=sq, in_=x_t, func=AF.Square, accum_out=ss)
        nrm = small.tile([P, 1], F32)
        nc.scalar.sqrt(nrm, ss)
        rstd = small.tile([P, 1], F32)
        nc.vector.reciprocal(rstd, nrm)
        xn_b = io_pool.tile([P, d], BF16)
        nc.vector.tensor_scalar_mul(xn_b, x_t, rstd)

        # ---- transpose xn -> lhsT for GEMM1 ----
        xnT = io_pool.tile([P, DT, P], BF16)
        for dd in range(DT):
            if USE_DMA_TRANSPOSE:
                nc.sync.dma_start(
                    xnT[:, dd, :], xn_b[:, dd * P:(dd + 1) * P], transpose=True
                )
            else:
                xps = psum_t.tile([P, P], BF16, tag="pst")
                nc.tensor.transpose(xps, xn_b[:, dd * P:(dd + 1) * P], ident)
                nc.scalar.copy(xnT[:, dd, :], xps)

        # ---- GEMM1 + exp ----
        exp_b = exp_pool.tile([P, k], BF16)
        parts = small.tile([P, NCH], F32)
        for n in range(NCH):
            ps = psum_g1.tile([P, 512], F32)
            for dd in range(DT):
                nc.tensor.matmul(
                    ps,
                    xnT[:, dd, :],
                    cb_t[:, dd, n * 512:(n + 1) * 512],
                    start=(dd == 0),
                    stop=(dd == DT - 1),
                )
            nc.scalar.activation(
                out=exp_b[:, n * 512:(n + 1) * 512],
                in_=ps,
                func=AF.Exp,
                scale=inv_tau,
                accum_out=parts[:, n:n + 1],
            )

        rs = small.tile([P, 1], F32)
        nc.vector.tensor_reduce(rs, parts, axis=mybir.AxisListType.X, op=ALU.add)
        rr = small.tile([P, 1], F32)
        nc.vector.reciprocal(rr, rs)

        # ---- GEMM2: q_unnorm = exp @ cb ----
        qacc = psum_q.tile([P, d], F32)
        for kk in range(KT):
            st = st_pool.tile([P, P], BF16)
            if USE_DMA_TRANSPOSE:
                nc.sync.dma_start(
                    st, exp_b[:, kk * P:(kk + 1) * P], transpose=True
                )
            else:
                pst = psum_t.tile([P, P], BF16, tag="pst")
                nc.tensor.transpose(pst, exp_b[:, kk * P:(kk + 1) * P], ident)
                nc.vector.tensor_copy(st, pst)
            nc.tensor.matmul(
                qacc, st, cb_n[:, kk, :], start=(kk == 0), stop=(kk == KT - 1)
            )

        # ---- epilogue: fold softmax denominator into output scale ----
        o_sb = io_pool.tile([P, d], F32)
        nc.vector.tensor_scalar_mul(o_sb, qacc, rr)
        nc.sync.dma_start(out_ap[row0:row0 + P, :], o_sb)


def build_bass(n_local, k=K_FULL, d=D_FULL, n_cores=N_CORES, reps=1):
    nc = bacc.Bacc(
        "TRN2",
        target_bir_lowering=False,
        debug=False,
        num_devices=n_cores,
    )
    x_ap = nc.dram_tensor("x", [n_local, d], F32, kind="ExternalInput").ap()
    cb_ap = nc.dram_tensor("codebook", [k, d], F32, kind="ExternalInput").ap()
    out_ap = nc.dram_tensor("out", [n_local, d], F32, kind="ExternalOutput").ap()
    with tile.TileContext(nc) as tc:
        _build_kernel(tc, out_ap, x_ap, cb_ap, n_local, k, d, reps=reps)
    nc.compile()
    return nc


_NC_CACHE = {}


def _get_nc(n_local, k, d, n_cores, reps=1):
    key = (n_local, k, d, n_cores, reps, USE_DMA_TRANSPOSE)
    if key not in _NC_CACHE:
        _NC_CACHE[key] = build_bass(n_local, k, d, n_cores, reps=reps)
    return _NC_CACHE[key]


def run_sharded(x, codebook, trace=False, reps=1):
    n, d = x.shape
    k = codebook.shape[0]
    assert n % N_CORES == 0
    n_local = n // N_CORES
    nc = _get_nc(n_local, k, d, N_CORES, reps=reps)
    cb = np.ascontiguousarray(codebook, dtype=np.float32)
    in_maps = [
        {
            "x": np.ascontiguousarray(x[i * n_local:(i + 1) * n_local],
                                      dtype=np.float32),
            "codebook": cb,
        }
        for i in range(N_CORES)
    ]
    res = run_bass_kernel_spmd(
        nc, in_maps, core_ids=list(range(N_CORES)), trace=trace
    )
    out = np.concatenate([r["out"] for r in res.results], axis=0)
    return out, res


def kernel(x, codebook):
    out, _ = run_sharded(x, codebook, trace=False)
    return out
